# revision 13
# baseline (speedup 1.0000x reference)
"""Trainium2 Bass kernel for AcceleratedAttentionPool1d.

Key algebraic insight: the reference materializes full [B,S,K,K] window
attention but only keeps the CENTER row (k=pad) of each window. So per
output position s we need:
  - Qtok = Wq @ xp + bq  (per-token query projection over padded seq)
  - energy[s, j] = <Qtok[:, s+4], Qtok[:, s0+j]> / 24 over a 9-wide band
  - attn = softmax over the 9 band entries
  - u[:, s] = sum_j attn[s, j] * xp[:, s0+j]   (V = raw window tokens)
  - out[:, s] = (Wo @ u + bo) / 9

Sharding: data-parallel over batch; B=8 batches on 8 cores, one each.

Per-core dataflow (S processed in 18 chunks of C=120 with halo H=128):
  banded energy = one [120,128] PE matmul pair per chunk; additive band
  mask; softmax with free-axis reductions (exp's accum_out gives row
  sums for free); one PE transpose of the attention tile; AV and output
  projections as PE matmuls with the output projection batched across
  groups of 4 chunks for bigger free dims. Bias adds are fused into
  ScalarE PSUM->SBUF evictions; /9 and transposes of weights/x are done
  on the host.

MODE selects matmul precision: 'fp32' (exact, PE double-pass),
'fp32r' (single-pass fp32), 'bf16' (bf16 operands).
"""

import numpy as np
import ml_dtypes

import concourse.bass as bass
import concourse.mybir as mybir
import concourse.tile as tile
from concourse import bacc
from concourse.bass import ts
from concourse.bass_utils import run_bass_kernel_spmd

F32 = mybir.dt.float32
BF16 = mybir.dt.bfloat16
F32R = mybir.dt.float32r

B, E, S = 8, 256, 2048
KERNEL = 9
PAD = KERNEL // 2
SP = S + 2 * PAD  # 2056
SCALE = 1.0 / (np.sqrt(E) * 1.5)  # 1/24
C = 120  # output positions per chunk
H = 128  # halo width (C + KERNEL - 1)
NCHUNK = 18  # 17 full strides + 1 overlapping tail chunk
GROUP = 4  # chunks per output-projection group
NEG = -1.0e30

MODE = "fp32"  # 'fp32' | 'fp32r' | 'bf16'

_T_CHUNKS = [(0, 512), (512, 512), (1024, 512), (1536, 512), (2048, 8)]


def _chunk_start(c: int) -> int:
    return 120 * c if c < NCHUNK - 1 else S - C  # last chunk overlaps


def _groups():
    """Yield lists of chunk indices per output-projection group."""
    out = []
    for g0 in range(0, NCHUNK, GROUP):
        out.append(list(range(g0, min(g0 + GROUP, NCHUNK))))
    return out


def build_nc(mode=None) -> bass.Bass:
    mode = mode or MODE
    # matmul-operand storage dtype
    mdt = {"bf16": BF16, "fp32r": F32R, "fp32": F32}[mode]

    def mm_ap(ap):
        return ap

    nc = bacc.Bacc("TRN2", target_bir_lowering=False)

    xp_d = nc.dram_tensor("xp", [E, SP], mdt, kind="ExternalInput")
    xpt_d = nc.dram_tensor("xpt", [SP, E], mdt, kind="ExternalInput")
    wqt_d = nc.dram_tensor("wqt", [E, E], mdt, kind="ExternalInput")
    wot_d = nc.dram_tensor("wot", [E, E], mdt, kind="ExternalInput")
    bq_d = nc.dram_tensor("bqv", [128, 2], F32, kind="ExternalInput")
    bo_d = nc.dram_tensor("bov", [128, 2], F32, kind="ExternalInput")
    mask_d = nc.dram_tensor("mask", [C, H], F32, kind="ExternalInput")
    id_d = nc.dram_tensor("ident", [128, 128], mdt, kind="ExternalInput")
    out_d = nc.dram_tensor("out", [E, S], F32, kind="ExternalOutput")

    with tile.TileContext(nc) as tc:
        with (
            tc.tile_pool(name="const", bufs=1) as const,
            tc.tile_pool(name="work", bufs=3) as work,
            tc.tile_pool(name="grp", bufs=2) as grp,
            tc.tile_pool(name="halo", bufs=3) as halo,
        ):
            # constants first: the first matmul needs wqt, and DMAs drain
            # in issue order on the sync queue
            wqt_t = const.tile([128, 2, E], mdt)
            nc.sync.dma_start(wqt_t, wqt_d[:, :].rearrange("(i p) f -> p i f", p=128))
            bq_t = const.tile([128, 2], F32)
            nc.sync.dma_start(bq_t, bq_d[:, :])
            mask_t = const.tile([C, H], F32)
            nc.sync.dma_start(mask_t, mask_d[:, :])
            id_t = const.tile([128, 128], mdt)
            nc.sync.dma_start(id_t, id_d[:, :])
            wot_t = const.tile([128, 2, E], mdt)
            nc.sync.dma_start(wot_t, wot_d[:, :].rearrange("(i p) f -> p i f", p=128))
            bo_t = const.tile([128, 2], F32)
            nc.sync.dma_start(bo_t, bo_d[:, :])
            xp_view = xp_d[:, :].rearrange("(i p) t -> p i t", p=128)
            xp_t = const.tile([128, 2, SP], mdt)
            for t0, w in _T_CHUNKS:
                for e_i in range(2):
                    nc.sync.dma_start(
                        xp_t[:, e_i, t0 : t0 + w], xp_view[:, e_i, t0 : t0 + w]
                    )

            qtok_t = const.tile([128, 2, SP], mdt)

            # Stage 1: Qtok[f, t] = sum_e Wq[f, e] xp[e, t] + bq[f]
            with tc.tile_pool(name="psq", bufs=2, space="PSUM") as psq:
                for t0, w in _T_CHUNKS:
                    for f_i in range(2):
                        pq = psq.tile([128, 512], F32)
                        for e_i in range(2):
                            nc.tensor.matmul(
                                pq[:, :w],
                                lhsT=mm_ap(wqt_t[:, e_i, ts(f_i, 128)]),
                                rhs=mm_ap(xp_t[:, e_i, t0 : t0 + w]),
                                start=(e_i == 0),
                                stop=(e_i == 1),
                            )
                        nc.scalar.activation(
                            qtok_t[:, f_i, t0 : t0 + w],
                            pq[:, :w],
                            mybir.ActivationFunctionType.Identity,
                            bias=bq_t[:, f_i : f_i + 1],
                            scale=1.0,
                        )

            out_view = out_d[:, :].rearrange("(i p) s -> p i s", p=128)

            with (
                tc.tile_pool(name="pse", bufs=2, space="PSUM") as pse,
                tc.tile_pool(name="psat", bufs=2, space="PSUM") as psat,
                tc.tile_pool(name="psu", bufs=2, space="PSUM") as psu,
                tc.tile_pool(name="psf", bufs=2, space="PSUM") as psf,
            ):
                for grp_chunks in _groups():
                    ng = len(grp_chunks)
                    gw = ng * C
                    ug = grp.tile([128, 2, gw], mdt, tag="ug")

                    # banded energies for the whole group into ONE psum bank:
                    # [C, ng, H]; each chunk is a [C, H] free-slice
                    pe_ = pse.tile([C, 4, H], F32)
                    for gi, c in enumerate(grp_chunks):
                        s0 = _chunk_start(c)
                        for f_i in range(2):
                            nc.tensor.matmul(
                                pe_[:, gi, :],
                                lhsT=mm_ap(
                                    qtok_t[:, f_i, s0 + PAD : s0 + PAD + C]
                                ),
                                rhs=mm_ap(qtok_t[:, f_i, s0 : s0 + H]),
                                start=(f_i == 0),
                                stop=(f_i == 1),
                            )
                    # fused group softmax: mask-add, exp, row sums, normalize
                    nc.vector.tensor_tensor(
                        out=pe_[:, :ng, :],
                        in0=pe_[:, :ng, :],
                        in1=mask_t[:, None, :].to_broadcast((C, ng, H)),
                        op=mybir.AluOpType.add,
                    )
                    A = work.tile([128, 4, H], mdt, tag="A")
                    nc.gpsimd.memset(A[96:128, :, :].bitcast(mybir.dt.uint32), 0)
                    nc.scalar.activation(
                        A[:C, :ng, :],
                        pe_[:, :ng, :],
                        mybir.ActivationFunctionType.Exp,
                        scale=SCALE,
                    )
                    sums = work.tile([C, 4], F32, tag="sums")
                    nc.vector.tensor_reduce(
                        sums[:, :ng],
                        A[:C, :ng, :],
                        axis=mybir.AxisListType.X,
                        op=mybir.AluOpType.add,
                    )
                    r = work.tile([C, 4], F32, tag="r")
                    nc.vector.reciprocal(r[:, :ng], sums[:, :ng])
                    nc.vector.tensor_tensor(
                        out=A[:C, :ng, :],
                        in0=A[:C, :ng, :],
                        in1=r[:, :ng, None].to_broadcast((C, ng, H)),
                        op=mybir.AluOpType.mult,
                    )
                    for gi, c in enumerate(grp_chunks):
                        s0 = _chunk_start(c)
                        # transpose attention to [H, C] for the AV matmul
                        pat = psat.tile([128, 128], mdt)
                        nc.tensor.transpose(pat, A[:, gi, :], id_t)
                        at = work.tile([128, 128], mdt, tag="at")
                        nc.vector.tensor_copy(at, pat)
                        # halo of x^T for the V side
                        xh = halo.tile([128, E], mdt, tag="xh")
                        nc.sync.dma_start(xh, xpt_d[s0 : s0 + H, :])
                        # u[e, s] = sum_j xpt[s0+j, e] * at[j, s]
                        pu = psu.tile([128, 2, C], F32)
                        for e_i in range(2):
                            nc.tensor.matmul(
                                pu[:, e_i, :],
                                lhsT=mm_ap(xh[:, ts(e_i, 128)]),
                                rhs=mm_ap(at[:, :C]),
                                start=True,
                                stop=True,
                            )
                        nc.scalar.copy(ug[:, :, gi * C : (gi + 1) * C], pu)

                    # fin[f, s] = sum_e (Wo/9)[f, e] u[e, s] + bo/9
                    fo = grp.tile([128, 2, gw], F32, tag="fo")
                    for f_i in range(2):
                        pf = psf.tile([128, 512], F32)
                        for e_i in range(2):
                            nc.tensor.matmul(
                                pf[:, :gw],
                                lhsT=mm_ap(wot_t[:, e_i, ts(f_i, 128)]),
                                rhs=mm_ap(ug[:, e_i, :]),
                                start=(e_i == 0),
                                stop=(e_i == 1),
                            )
                        nc.scalar.activation(
                            fo[:, f_i, :],
                            pf[:, :gw],
                            mybir.ActivationFunctionType.Identity,
                            bias=bo_t[:, f_i : f_i + 1],
                            scale=1.0,
                        )
                    # store: non-tail chunks in a group are contiguous in S,
                    # so they go out as one DMA; the overlapping tail chunk
                    # contributes only its last 8 columns
                    plain = [c for c in grp_chunks if c < NCHUNK - 1]
                    if plain:
                        s0 = _chunk_start(plain[0])
                        nc.sync.dma_start(
                            out_view[:, :, s0 : s0 + len(plain) * C],
                            fo[:, :, : len(plain) * C],
                        )
                    if grp_chunks[-1] == NCHUNK - 1:
                        gi = len(grp_chunks) - 1
                        s0 = _chunk_start(NCHUNK - 1)
                        d0 = 120 * (NCHUNK - 1) - s0  # 112
                        nc.sync.dma_start(
                            out_view[:, :, s0 + d0 : s0 + C],
                            fo[:, :, gi * C + d0 : (gi + 1) * C],
                        )
    nc.compile()
    return nc


def make_in_maps(x, Wq, bq, Wo, bo, mode=None):
    mode = mode or MODE
    npdt = ml_dtypes.bfloat16 if mode == "bf16" else np.float32

    x = np.asarray(x, dtype=np.float32)
    Wq = np.asarray(Wq, dtype=np.float32)
    bq = np.asarray(bq, dtype=np.float32)
    Wo = np.asarray(Wo, dtype=np.float32)
    bo = np.asarray(bo, dtype=np.float32)

    wqt = np.ascontiguousarray(Wq.T).astype(npdt)
    wot = np.ascontiguousarray((Wo / KERNEL).T).astype(npdt)
    bqv = np.ascontiguousarray(bq.reshape(2, 128).T)
    bov = np.ascontiguousarray((bo / KERNEL).reshape(2, 128).T)

    mask = np.full((C, H), NEG, dtype=np.float32)
    for m in range(C):
        mask[m, m : m + KERNEL] = 0.0
    ident = np.eye(128, dtype=npdt)

    in_maps = []
    for b in range(B):
        xp = np.zeros((E, SP), dtype=np.float32)
        xp[:, PAD : PAD + S] = x[b]
        xpt = np.ascontiguousarray(xp.T).astype(npdt)
        in_maps.append(
            dict(
                xp=xp.astype(npdt),
                xpt=xpt,
                wqt=wqt,
                wot=wot,
                bqv=bqv,
                bov=bov,
                mask=mask,
                ident=ident,
            )
        )
    return in_maps


def kernel(x, Wq, bq, Wo, bo):
    res = kernel_with_results(x, Wq, bq, Wo, bo)
    return np.stack([r["out"] for r in res.results]).astype(np.float32)


def kernel_with_results(x, Wq, bq, Wo, bo, trace=False, mode=None, **kwargs):
    in_maps = make_in_maps(x, Wq, bq, Wo, bo, mode=mode)
    nc = build_nc(mode=mode)
    return run_bass_kernel_spmd(
        nc, in_maps, core_ids=list(range(B)), trace=trace, **kwargs
    )


# revision 15
# speedup vs baseline: 1.0196x; 1.0196x over previous
"""Trainium2 Bass kernel for AcceleratedAttentionPool1d.

Key algebraic insight: the reference materializes full [B,S,K,K] window
attention but only keeps the CENTER row (k=pad) of each window. So per
output position s we need:
  - Qtok = Wq @ xp + bq  (per-token query projection over padded seq)
  - energy[s, j] = <Qtok[:, s+4], Qtok[:, s0+j]> / 24 over a 9-wide band
  - attn = softmax over the 9 band entries
  - u[:, s] = sum_j attn[s, j] * xp[:, s0+j]   (V = raw window tokens)
  - out[:, s] = (Wo @ u + bo) / 9

Sharding: data-parallel over batch; B=8 batches on 8 cores, one each.

Per-core dataflow (S processed in 18 chunks of C=120 with halo H=128):
  banded energy = one [120,128] PE matmul pair per chunk; additive band
  mask; softmax with free-axis reductions (exp's accum_out gives row
  sums for free); one PE transpose of the attention tile; AV and output
  projections as PE matmuls with the output projection batched across
  groups of 4 chunks for bigger free dims. Bias adds are fused into
  ScalarE PSUM->SBUF evictions; /9 and transposes of weights/x are done
  on the host.

MODE selects matmul precision: 'fp32' (exact, PE double-pass),
'fp32r' (single-pass fp32), 'bf16' (bf16 operands).
"""

import numpy as np
import ml_dtypes

import concourse.bass as bass
import concourse.mybir as mybir
import concourse.tile as tile
from concourse import bacc
from concourse.bass import ts
from concourse.bass_utils import run_bass_kernel_spmd

F32 = mybir.dt.float32
BF16 = mybir.dt.bfloat16
F32R = mybir.dt.float32r

B, E, S = 8, 256, 2048
KERNEL = 9
PAD = KERNEL // 2
SP = S + 2 * PAD  # 2056
SCALE = 1.0 / (np.sqrt(E) * 1.5)  # 1/24
C = 120  # output positions per chunk
H = 128  # halo width (C + KERNEL - 1)
NCHUNK = 18  # 17 full strides + 1 overlapping tail chunk
GROUP = 4  # chunks per output-projection group
SOFT_G = 2  # chunks per fused-softmax subgroup (must divide GROUP)
NEG = -1.0e30

MODE = "fp32"  # 'fp32' | 'fp32r' | 'bf16'

_T_CHUNKS = [(0, 512), (512, 512), (1024, 512), (1536, 512), (2048, 8)]


def _chunk_start(c: int) -> int:
    return 120 * c if c < NCHUNK - 1 else S - C  # last chunk overlaps


def _groups():
    """Yield lists of chunk indices per output-projection group."""
    out = []
    for g0 in range(0, NCHUNK, GROUP):
        out.append(list(range(g0, min(g0 + GROUP, NCHUNK))))
    return out


def build_nc(mode=None) -> bass.Bass:
    mode = mode or MODE
    # matmul-operand storage dtype
    mdt = {"bf16": BF16, "fp32r": F32R, "fp32": F32}[mode]

    def mm_ap(ap):
        return ap

    nc = bacc.Bacc("TRN2", target_bir_lowering=False)

    xp_d = nc.dram_tensor("xp", [E, SP], mdt, kind="ExternalInput")
    xpt_d = nc.dram_tensor("xpt", [SP, E], mdt, kind="ExternalInput")
    wqt_d = nc.dram_tensor("wqt", [E, E], mdt, kind="ExternalInput")
    wot_d = nc.dram_tensor("wot", [E, E], mdt, kind="ExternalInput")
    bq_d = nc.dram_tensor("bqv", [128, 2], F32, kind="ExternalInput")
    bo_d = nc.dram_tensor("bov", [128, 2], F32, kind="ExternalInput")
    mask_d = nc.dram_tensor("mask", [C, H], F32, kind="ExternalInput")
    id_d = nc.dram_tensor("ident", [128, 128], mdt, kind="ExternalInput")
    out_d = nc.dram_tensor("out", [E, S], F32, kind="ExternalOutput")

    with tile.TileContext(nc) as tc:
        with (
            tc.tile_pool(name="const", bufs=1) as const,
            tc.tile_pool(name="work", bufs=3) as work,
            tc.tile_pool(name="grp", bufs=2) as grp,
            tc.tile_pool(name="halo", bufs=3) as halo,
        ):
            # constants first: the first matmul needs wqt, and DMAs drain
            # in issue order on the sync queue
            wqt_t = const.tile([128, 2, E], mdt)
            nc.sync.dma_start(wqt_t, wqt_d[:, :].rearrange("(i p) f -> p i f", p=128))
            bq_t = const.tile([128, 2], F32)
            nc.sync.dma_start(bq_t, bq_d[:, :])
            mask_t = const.tile([C, H], F32)
            nc.sync.dma_start(mask_t, mask_d[:, :])
            id_t = const.tile([128, 128], mdt)
            nc.sync.dma_start(id_t, id_d[:, :])
            wot_t = const.tile([128, 2, E], mdt)
            nc.sync.dma_start(wot_t, wot_d[:, :].rearrange("(i p) f -> p i f", p=128))
            bo_t = const.tile([128, 2], F32)
            nc.sync.dma_start(bo_t, bo_d[:, :])
            xp_view = xp_d[:, :].rearrange("(i p) t -> p i t", p=128)
            xp_t = const.tile([128, 2, SP], mdt)
            for t0, w in _T_CHUNKS:
                for e_i in range(2):
                    nc.sync.dma_start(
                        xp_t[:, e_i, t0 : t0 + w], xp_view[:, e_i, t0 : t0 + w]
                    )

            qtok_t = const.tile([128, 2, SP], mdt)

            # Stage 1: Qtok[f, t] = sum_e Wq[f, e] xp[e, t] + bq[f]
            with tc.tile_pool(name="psq", bufs=2, space="PSUM") as psq:
                for t0, w in _T_CHUNKS:
                    for f_i in range(2):
                        pq = psq.tile([128, 512], F32)
                        for e_i in range(2):
                            nc.tensor.matmul(
                                pq[:, :w],
                                lhsT=mm_ap(wqt_t[:, e_i, ts(f_i, 128)]),
                                rhs=mm_ap(xp_t[:, e_i, t0 : t0 + w]),
                                start=(e_i == 0),
                                stop=(e_i == 1),
                            )
                        nc.scalar.activation(
                            qtok_t[:, f_i, t0 : t0 + w],
                            pq[:, :w],
                            mybir.ActivationFunctionType.Identity,
                            bias=bq_t[:, f_i : f_i + 1],
                            scale=1.0,
                        )

            out_view = out_d[:, :].rearrange("(i p) s -> p i s", p=128)

            with (
                tc.tile_pool(name="pse", bufs=2, space="PSUM") as pse,
                tc.tile_pool(name="psat", bufs=2, space="PSUM") as psat,
                tc.tile_pool(name="psu", bufs=2, space="PSUM") as psu,
                tc.tile_pool(name="psf", bufs=2, space="PSUM") as psf,
            ):
                for grp_chunks in _groups():
                    ng = len(grp_chunks)
                    gw = ng * C
                    ug = grp.tile([128, 2, gw], mdt, tag="ug")

                    for sg0 in range(0, ng, SOFT_G):
                        sg_chunks = grp_chunks[sg0 : sg0 + SOFT_G]
                        sg = len(sg_chunks)
                        # banded energies for the subgroup into ONE psum
                        # bank: [C, sg, H]; each chunk a [C, H] free-slice
                        pe_ = pse.tile([C, SOFT_G, H], F32)
                        for gi, c in enumerate(sg_chunks):
                            s0 = _chunk_start(c)
                            for f_i in range(2):
                                nc.tensor.matmul(
                                    pe_[:, gi, :],
                                    lhsT=mm_ap(
                                        qtok_t[:, f_i, s0 + PAD : s0 + PAD + C]
                                    ),
                                    rhs=mm_ap(qtok_t[:, f_i, s0 : s0 + H]),
                                    start=(f_i == 0),
                                    stop=(f_i == 1),
                                )
                        # fused subgroup softmax
                        nc.vector.tensor_tensor(
                            out=pe_[:, :sg, :],
                            in0=pe_[:, :sg, :],
                            in1=mask_t[:, None, :].to_broadcast((C, sg, H)),
                            op=mybir.AluOpType.add,
                        )
                        A = work.tile([128, SOFT_G, H], mdt, tag="A")
                        nc.gpsimd.memset(
                            A[96:128, :, :].bitcast(mybir.dt.uint32), 0
                        )
                        nc.scalar.activation(
                            A[:C, :sg, :],
                            pe_[:, :sg, :],
                            mybir.ActivationFunctionType.Exp,
                            scale=SCALE,
                        )
                        sums = work.tile([C, SOFT_G], F32, tag="sums")
                        nc.vector.tensor_reduce(
                            sums[:, :sg],
                            A[:C, :sg, :],
                            axis=mybir.AxisListType.X,
                            op=mybir.AluOpType.add,
                        )
                        r = work.tile([C, SOFT_G], F32, tag="r")
                        nc.vector.reciprocal(r[:, :sg], sums[:, :sg])
                        nc.vector.tensor_tensor(
                            out=A[:C, :sg, :],
                            in0=A[:C, :sg, :],
                            in1=r[:, :sg, None].to_broadcast((C, sg, H)),
                            op=mybir.AluOpType.mult,
                        )
                        for gi, c in enumerate(sg_chunks):
                            s0 = _chunk_start(c)
                            # transpose attention to [H, C] for AV matmul
                            pat = psat.tile([128, 128], mdt)
                            nc.tensor.transpose(pat, A[:, gi, :], id_t)
                            at = work.tile([128, 128], mdt, tag="at")
                            nc.vector.tensor_copy(at, pat)
                            # halo of x^T for the V side (2nd HWDGE ring)
                            xh = halo.tile([128, E], mdt, tag="xh")
                            nc.scalar.dma_start(xh, xpt_d[s0 : s0 + H, :])
                            # u[e, s] = sum_j xpt[s0+j, e] * at[j, s]
                            pu = psu.tile([128, 2, C], F32)
                            for e_i in range(2):
                                nc.tensor.matmul(
                                    pu[:, e_i, :],
                                    lhsT=mm_ap(xh[:, ts(e_i, 128)]),
                                    rhs=mm_ap(at[:, :C]),
                                    start=True,
                                    stop=True,
                                )
                            nc.scalar.copy(
                                ug[:, :, (sg0 + gi) * C : (sg0 + gi + 1) * C], pu
                            )

                    # fin[f, s] = sum_e (Wo/9)[f, e] u[e, s] + bo/9
                    fo = grp.tile([128, 2, gw], F32, tag="fo")
                    for f_i in range(2):
                        pf = psf.tile([128, 512], F32)
                        for e_i in range(2):
                            nc.tensor.matmul(
                                pf[:, :gw],
                                lhsT=mm_ap(wot_t[:, e_i, ts(f_i, 128)]),
                                rhs=mm_ap(ug[:, e_i, :]),
                                start=(e_i == 0),
                                stop=(e_i == 1),
                            )
                        nc.scalar.activation(
                            fo[:, f_i, :],
                            pf[:, :gw],
                            mybir.ActivationFunctionType.Identity,
                            bias=bo_t[:, f_i : f_i + 1],
                            scale=1.0,
                        )
                    # store: non-tail chunks in a group are contiguous in S,
                    # so they go out as one DMA; the overlapping tail chunk
                    # contributes only its last 8 columns
                    plain = [c for c in grp_chunks if c < NCHUNK - 1]
                    if plain:
                        s0 = _chunk_start(plain[0])
                        nc.sync.dma_start(
                            out_view[:, :, s0 : s0 + len(plain) * C],
                            fo[:, :, : len(plain) * C],
                        )
                    if grp_chunks[-1] == NCHUNK - 1:
                        gi = len(grp_chunks) - 1
                        s0 = _chunk_start(NCHUNK - 1)
                        d0 = 120 * (NCHUNK - 1) - s0  # 112
                        nc.sync.dma_start(
                            out_view[:, :, s0 + d0 : s0 + C],
                            fo[:, :, gi * C + d0 : (gi + 1) * C],
                        )
    nc.compile()
    return nc


def make_in_maps(x, Wq, bq, Wo, bo, mode=None):
    mode = mode or MODE
    npdt = ml_dtypes.bfloat16 if mode == "bf16" else np.float32

    x = np.asarray(x, dtype=np.float32)
    Wq = np.asarray(Wq, dtype=np.float32)
    bq = np.asarray(bq, dtype=np.float32)
    Wo = np.asarray(Wo, dtype=np.float32)
    bo = np.asarray(bo, dtype=np.float32)

    wqt = np.ascontiguousarray(Wq.T).astype(npdt)
    wot = np.ascontiguousarray((Wo / KERNEL).T).astype(npdt)
    bqv = np.ascontiguousarray(bq.reshape(2, 128).T)
    bov = np.ascontiguousarray((bo / KERNEL).reshape(2, 128).T)

    mask = np.full((C, H), NEG, dtype=np.float32)
    for m in range(C):
        mask[m, m : m + KERNEL] = 0.0
    ident = np.eye(128, dtype=npdt)

    in_maps = []
    for b in range(B):
        xp = np.zeros((E, SP), dtype=np.float32)
        xp[:, PAD : PAD + S] = x[b]
        xpt = np.ascontiguousarray(xp.T).astype(npdt)
        in_maps.append(
            dict(
                xp=xp.astype(npdt),
                xpt=xpt,
                wqt=wqt,
                wot=wot,
                bqv=bqv,
                bov=bov,
                mask=mask,
                ident=ident,
            )
        )
    return in_maps


def kernel(x, Wq, bq, Wo, bo):
    res = kernel_with_results(x, Wq, bq, Wo, bo)
    return np.stack([r["out"] for r in res.results]).astype(np.float32)


def kernel_with_results(x, Wq, bq, Wo, bo, trace=False, mode=None, **kwargs):
    in_maps = make_in_maps(x, Wq, bq, Wo, bo, mode=mode)
    nc = build_nc(mode=mode)
    return run_bass_kernel_spmd(
        nc, in_maps, core_ids=list(range(B)), trace=trace, **kwargs
    )
